# revision 1
# baseline (speedup 1.0000x reference)
"""Trainium2 Bass kernel for nn_LSTMClassifier (B=128, T=2048, I=6, H=256).

Strategy
--------
Pure data parallel: batch 128 is sharded 16 per NeuronCore across 8 cores;
all weights are replicated.  The LSTM recurrence is computed in a fully
"transposed" layout so every engine runs at full 128-partition width:

  * gates live as gates^T chunks: PSUM tiles [128 gate-dims, 16 batch]
    with chunk order [g0,g1 | i0,i1,f0,f1 | o0,o1] (gate-dim permutation
    is applied to the weights on the host, so tanh/sigmoid each cover
    contiguous column blocks).
  * per step, per gate-chunk: 3 accumulating matmuls
      W_hh^T[k-chunk0] @ h0  +  W_hh^T[k-chunk1] @ h1  +  W_xb^T @ [x_t;1]
    where the stationary operands are bf16 (fast weight load) and the
    moving operands are the 16-column h / x vectors.
  * c and h states are [128, 32] tiles (H-dim on partitions, batch x
    2 H-chunks on free dim), so the freshly computed h tile IS the next
    step's matmul moving operand - no transposes anywhere in the loop.
  * biases are folded into row 6 of the augmented input projection
    (the xa input carries a constant 1.0 row).

The classification heads are two small matmuls on the final h plus a bias
add, evaluated once after the time loop.
"""

from contextlib import ExitStack

import ml_dtypes
import numpy as np

import concourse.bass as bass
import concourse.mybir as mybir
import concourse.tile as tile
from concourse import bacc
from concourse.bass import ds, ts
from concourse.bass_utils import run_bass_kernel_spmd

B, T, I, H = 128, 2048, 6, 256
NCORES = 8
BS = B // NCORES  # 16 batch rows per core
NCH = 8  # gate chunks of 128
UNROLL = 16  # steps per For_i body

BF16 = mybir.dt.bfloat16
F32 = mybir.dt.float32
AF = mybir.ActivationFunctionType
NPBF16 = ml_dtypes.bfloat16

# gate-dim permutation: torch gate order is [i, f, g, o]; we reorder rows to
# [g, i, f, o] so tanh (g) and sigmoid (i,f) cover contiguous chunks.
PERM = np.r_[512:768, 0:256, 256:512, 768:1024]


def build(t_steps: int = T, unroll: int = UNROLL):
    """Build + compile the single-core Bass program (same program on all 8)."""
    nc = bacc.Bacc(
        "TRN2", target_bir_lowering=False, debug=False, num_devices=NCORES
    )
    whh_d = nc.dram_tensor("whh", [128, 16 * 128], BF16, kind="ExternalInput")
    wxb_d = nc.dram_tensor("wxb", [7, NCH * 128], BF16, kind="ExternalInput")
    xa_d = nc.dram_tensor("xa", [7, t_steps * BS], BF16, kind="ExternalInput")
    whd_d = nc.dram_tensor("whd", [128, 20], BF16, kind="ExternalInput")
    bhd_d = nc.dram_tensor("bhd", [BS, 10], F32, kind="ExternalInput")
    out_d = nc.dram_tensor("out", [BS, 10], F32, kind="ExternalOutput")

    with tile.TileContext(nc) as tc, ExitStack() as ctx:
        const = ctx.enter_context(tc.tile_pool(name="const", bufs=1))
        state = ctx.enter_context(tc.tile_pool(name="state", bufs=1))
        work = ctx.enter_context(tc.tile_pool(name="work", bufs=2))
        psum = ctx.enter_context(
            tc.tile_pool(name="psum", bufs=2, space=bass.MemorySpace.PSUM)
        )

        whh_sb = const.tile([128, 16 * 128], BF16, tag="whh")
        nc.sync.dma_start(whh_sb[:], whh_d[:])
        wxb_sb = const.tile([7, NCH * 128], BF16, tag="wxb")
        nc.sync.dma_start(wxb_sb[:], wxb_d[:])
        xa_sb = const.tile([7, t_steps * BS], BF16, tag="xa")
        nc.sync.dma_start(xa_sb[:], xa_d[:])
        whd_sb = const.tile([128, 20], BF16, tag="whd")
        nc.sync.dma_start(whd_sb[:], whd_d[:])
        bhd_sb = const.tile([BS, 10], F32, tag="bhd")
        nc.sync.dma_start(bhd_sb[:], bhd_d[:])

        # ping-pong states: [128 partitions = H%128, 32 free = (H//128, batch)]
        hA = state.tile([128, 2 * BS], BF16, tag="hA")
        hB = state.tile([128, 2 * BS], BF16, tag="hB")
        cA = state.tile([128, 2 * BS], F32, tag="cA")
        cB = state.tile([128, 2 * BS], F32, tag="cB")
        nc.vector.memset(hA[:], 0.0)
        nc.vector.memset(cA[:], 0.0)

        def step(xoff, u, hsrc, csrc, hdst, cdst):
            # separate PSUM tiles per gate group so ACT reads don't collide
            # with PE writes in the same bank
            pg = psum.tile([128, 2 * BS], F32, tag="pg")
            pif = psum.tile([128, 4 * BS], F32, tag="pif")
            po = psum.tile([128, 2 * BS], F32, tag="po")
            targets = [(pg, 0), (pg, 1), (pif, 0), (pif, 1), (pif, 2), (pif, 3),
                       (po, 0), (po, 1)]
            xslice = xa_sb[:, ds(xoff + u * BS, BS)]
            for ci, (pt, sub) in enumerate(targets):
                dst = pt[:, ts(sub, BS)]
                nc.tensor.matmul(dst, whh_sb[:, ts(2 * ci, 128)],
                                 hsrc[:, 0:BS], start=True, stop=False)
                nc.tensor.matmul(dst, whh_sb[:, ts(2 * ci + 1, 128)],
                                 hsrc[:, BS:2 * BS], start=False, stop=False)
                nc.tensor.matmul(dst, wxb_sb[:, ts(ci, 128)], xslice,
                                 start=False, stop=True)
            tg = work.tile([128, 2 * BS], F32, tag="tg")
            nc.scalar.activation(tg[:], pg[:], AF.Tanh)
            sif = work.tile([128, 4 * BS], F32, tag="sif")
            nc.scalar.activation(sif[:], pif[:], AF.Sigmoid)
            so = work.tile([128, 2 * BS], F32, tag="so")
            nc.scalar.activation(so[:], po[:], AF.Sigmoid)
            ig = work.tile([128, 2 * BS], F32, tag="ig")
            nc.vector.tensor_mul(ig[:], sif[:, 0:2 * BS], tg[:])
            fc = work.tile([128, 2 * BS], F32, tag="fc")
            nc.vector.tensor_mul(fc[:], sif[:, 2 * BS:4 * BS], csrc[:])
            nc.vector.tensor_add(cdst[:], ig[:], fc[:])
            tcb = work.tile([128, 2 * BS], F32, tag="tcb")
            nc.scalar.activation(tcb[:], cdst[:], AF.Tanh)
            nc.vector.tensor_mul(hdst[:], so[:], tcb[:])

        assert t_steps % unroll == 0 and unroll % 2 == 0
        with tc.For_i(0, t_steps * BS, unroll * BS,
                      hint_engines=(mybir.EngineType.PE,)) as xoff:
            for u in range(unroll):
                if u % 2 == 0:
                    step(xoff, u, hA, cA, hB, cB)
                else:
                    step(xoff, u, hB, cB, hA, cA)

        # heads: out[b, j] = sum_k h[b, k] * Wcat[j, k] + bias[j]
        ph = psum.tile([BS, 10], F32, tag="ph")
        nc.tensor.matmul(ph[:], hA[:, 0:BS], whd_sb[:, 0:10],
                         start=True, stop=False)
        nc.tensor.matmul(ph[:], hA[:, BS:2 * BS], whd_sb[:, 10:20],
                         start=False, stop=True)
        outsb = work.tile([BS, 10], F32, tag="outsb")
        nc.vector.tensor_add(outsb[:], ph[:], bhd_sb[:])
        nc.sync.dma_start(out_d[:], outsb[:])

    nc.compile()
    return nc


def prep_weights(W_ih, W_hh, b_ih, b_hh, W_gender, b_gender, W_hand, b_hand,
                 W_years, b_years, W_level, b_level):
    W_ih = np.asarray(W_ih, np.float32)
    W_hh = np.asarray(W_hh, np.float32)
    bias = np.asarray(b_ih, np.float32) + np.asarray(b_hh, np.float32)

    Whh_p = W_hh[PERM, :]  # [1024, 256]
    # whh[k, (2c+kc)*128 + m] = Whh_p[128c+m, 128kc+k]
    whh = (Whh_p.reshape(NCH, 128, 2, 128).transpose(3, 0, 2, 1)
           .reshape(128, 16 * 128).astype(NPBF16))
    wxb = np.empty((7, NCH * 128), np.float32)
    wxb[0:6] = W_ih[PERM, :].T
    wxb[6] = bias[PERM]
    wxb = wxb.astype(NPBF16)

    Wcat = np.vstack([np.asarray(W_gender, np.float32),
                      np.asarray(W_hand, np.float32),
                      np.asarray(W_years, np.float32),
                      np.asarray(W_level, np.float32)])  # [10, 256]
    whd = np.empty((128, 20), np.float32)
    whd[:, 0:10] = Wcat[:, 0:128].T
    whd[:, 10:20] = Wcat[:, 128:256].T
    whd = whd.astype(NPBF16)
    bcat = np.concatenate([np.asarray(b_gender, np.float32),
                           np.asarray(b_hand, np.float32),
                           np.asarray(b_years, np.float32),
                           np.asarray(b_level, np.float32)])
    bhd = np.tile(bcat[None, :], (BS, 1)).astype(np.float32)
    return whh, wxb, whd, bhd


def prep_x(x_shard, t_steps):
    """x_shard [BS, t, 6] -> xa [7, t*BS] bf16 with constant-1 row."""
    xs = np.asarray(x_shard, np.float32)
    xa = np.empty((7, t_steps * BS), np.float32)
    xa[0:6] = xs.transpose(2, 1, 0).reshape(6, t_steps * BS)
    xa[6] = 1.0
    return xa.astype(NPBF16)


_CACHE = {}


def _get_nc():
    if "nc" not in _CACHE:
        _CACHE["nc"] = build()
    return _CACHE["nc"]


def kernel(x, W_ih, W_hh, b_ih, b_hh, W_gender, b_gender, W_hand, b_hand,
           W_years, b_years, W_level, b_level, _trace=False):
    nc = _get_nc()
    whh, wxb, whd, bhd = prep_weights(
        W_ih, W_hh, b_ih, b_hh, W_gender, b_gender, W_hand, b_hand,
        W_years, b_years, W_level, b_level)
    x = np.asarray(x, np.float32)
    in_maps = []
    for c in range(NCORES):
        xa = prep_x(x[c * BS:(c + 1) * BS], T)
        in_maps.append({"whh": whh, "wxb": wxb, "xa": xa, "whd": whd,
                        "bhd": bhd})
    res = run_bass_kernel_spmd(nc, in_maps, core_ids=list(range(NCORES)),
                               trace=_trace)
    _CACHE["last_results"] = res
    full = np.concatenate([res.results[i]["out"] for i in range(NCORES)],
                          axis=0).astype(np.float32)
    return (full[:, 0:2], full[:, 2:4], full[:, 4:7], full[:, 7:10])


# revision 7
# speedup vs baseline: 1.4677x; 1.4677x over previous
"""Trainium2 Bass kernel for nn_LSTMClassifier (B=128, T=2048, I=6, H=256).

Strategy
--------
Pure data parallel: batch 128 is sharded 16 per NeuronCore across 8 cores;
all weights are replicated.  The LSTM recurrence is computed in a fully
"transposed" layout so every engine runs at full 128-partition width:

  * gates live as gates^T chunks: PSUM tiles [128 gate-dims, 16 batch]
    with chunk order [g0,g1 | i0,i1,f0,f1 | o0,o1] (gate-dim permutation
    is applied to the weights on the host, so tanh/sigmoid each cover
    contiguous column blocks).
  * per step, per gate-chunk: 3 accumulating matmuls
      W_hh^T[k-chunk0] @ h0  +  W_hh^T[k-chunk1] @ h1  +  W_xb^T @ [x_t;1]
    where the stationary operands are bf16 (fast weight load) and the
    moving operands are the 16-column h / x vectors.
  * c and h states are [128, 32] tiles (H-dim on partitions, batch x
    2 H-chunks on free dim), so the freshly computed h tile IS the next
    step's matmul moving operand - no transposes anywhere in the loop.
  * biases are folded into row 6 of the augmented input projection
    (the xa input carries a constant 1.0 row).

The classification heads are two small matmuls on the final h plus a bias
add, evaluated once after the time loop.
"""

from contextlib import ExitStack

import ml_dtypes
import numpy as np

import concourse.bass as bass
import concourse.mybir as mybir
import concourse.tile as tile
from concourse import bacc
from concourse.bass import ds, ts
from concourse.bass_utils import run_bass_kernel_spmd

B, T, I, H = 128, 2048, 6, 256
NCORES = 8
BS = B // NCORES  # 16 batch rows per core
NCH = 8  # gate chunks of 128
UNROLL = 16  # steps per For_i body

BF16 = mybir.dt.bfloat16
F32 = mybir.dt.float32
AF = mybir.ActivationFunctionType
NPBF16 = ml_dtypes.bfloat16

# gate-dim permutation: torch gate order is [i, f, g, o]; we reorder rows to
# [g, i, f, o] so tanh (g) and sigmoid (i,f) cover contiguous chunks.
PERM = np.r_[512:768, 0:256, 256:512, 768:1024]


def build(t_steps: int = T, unroll: int = UNROLL):
    """Build + compile the single-core Bass program (same program on all 8)."""
    nc = bacc.Bacc(
        "TRN2", target_bir_lowering=False, debug=False, num_devices=NCORES
    )
    whh_d = nc.dram_tensor("whh", [128, 16 * 128], BF16, kind="ExternalInput")
    wxb_d = nc.dram_tensor("wxb", [7, NCH * 128], BF16, kind="ExternalInput")
    xa_d = nc.dram_tensor("xa", [7, t_steps * BS], BF16, kind="ExternalInput")
    whd_d = nc.dram_tensor("whd", [128, 20], BF16, kind="ExternalInput")
    bhd_d = nc.dram_tensor("bhd", [BS, 10], F32, kind="ExternalInput")
    out_d = nc.dram_tensor("out", [BS, 10], F32, kind="ExternalOutput")

    with tile.TileContext(nc) as tc, ExitStack() as ctx:
        const = ctx.enter_context(tc.tile_pool(name="const", bufs=1))
        state = ctx.enter_context(tc.tile_pool(name="state", bufs=1))
        work = ctx.enter_context(tc.tile_pool(name="work", bufs=2))
        psum = ctx.enter_context(
            tc.tile_pool(name="psum", bufs=2, space=bass.MemorySpace.PSUM)
        )

        whh_sb = const.tile([128, 16 * 128], BF16, tag="whh")
        nc.sync.dma_start(whh_sb[:], whh_d[:])
        wxb_sb = const.tile([7, NCH * 128], BF16, tag="wxb")
        nc.sync.dma_start(wxb_sb[:], wxb_d[:])
        xpool = ctx.enter_context(tc.tile_pool(name="xblk", bufs=2))
        whd_sb = const.tile([128, 20], BF16, tag="whd")
        nc.sync.dma_start(whd_sb[:], whd_d[:])
        bhd_sb = const.tile([BS, 10], F32, tag="bhd")
        nc.sync.dma_start(bhd_sb[:], bhd_d[:])

        # ping-pong states: [128 partitions = H%128, 32 free = (H//128, batch)]
        # c tiles carry a tanh(g) scratch slot in cols 0:32 so that
        # [i|f] (x) [tg|c] runs as ONE tensor_tensor op; c itself is cols 32:64
        hA = state.tile([128, 2 * BS], BF16, tag="hA")
        hB = state.tile([128, 2 * BS], BF16, tag="hB")
        cA = state.tile([128, 4 * BS], F32, tag="cA")
        cB = state.tile([128, 4 * BS], F32, tag="cB")
        nc.vector.memset(hA[:], 0.0)
        nc.vector.memset(cA[:], 0.0)

        # warm the ACT sigmoid table set before the loop so walrus doesn't
        # reload table sets on every iteration (sigmoid's set also has tanh)
        warm = work.tile([1, 1], F32, tag="warm")
        nc.vector.memset(warm[:], 0.0)
        nc.scalar.activation(warm[:], warm[:], AF.Sigmoid)
        nc.scalar.activation(warm[:], warm[:], AF.Tanh)

        def step(xt, u, hsrc, csrc, hdst, cdst):
            # separate PSUM tiles per gate group so ACT reads don't collide
            # with PE writes in the same bank
            pg = psum.tile([128, 2 * BS], F32, tag="pg")
            pif = psum.tile([128, 4 * BS], F32, tag="pif")
            po = psum.tile([128, 2 * BS], F32, tag="po")
            targets = [(pg, 0), (pg, 1), (pif, 0), (pif, 1), (pif, 2), (pif, 3),
                       (po, 0), (po, 1)]
            xslice = xt[:, ts(u, BS)]
            for ci, (pt, sub) in enumerate(targets):
                dst = pt[:, ts(sub, BS)]
                nc.tensor.matmul(dst, whh_sb[:, ts(2 * ci, 128)],
                                 hsrc[:, 0:BS], start=True, stop=False)
                nc.tensor.matmul(dst, whh_sb[:, ts(2 * ci + 1, 128)],
                                 hsrc[:, BS:2 * BS], start=False, stop=False)
                nc.tensor.matmul(dst, wxb_sb[:, ts(ci, 128)], xslice,
                                 start=False, stop=True)
            nc.scalar.activation(csrc[:, 0:2 * BS], pg[:], AF.Tanh)  # tg slot
            sif = work.tile([128, 4 * BS], F32, tag="sif")
            nc.scalar.activation(sif[:], pif[:], AF.Sigmoid)
            so = work.tile([128, 2 * BS], F32, tag="so")
            nc.scalar.activation(so[:], po[:], AF.Sigmoid)
            # [ig | fc] = [si | sf] * [tg | c_prev] in one op
            igfc = work.tile([128, 4 * BS], F32, tag="igfc")
            nc.vector.tensor_mul(igfc[:], sif[:], csrc[:])
            nc.vector.tensor_add(cdst[:, 2 * BS:4 * BS], igfc[:, 0:2 * BS],
                                 igfc[:, 2 * BS:4 * BS])
            tcb = work.tile([128, 2 * BS], F32, tag="tcb")
            nc.scalar.activation(tcb[:], cdst[:, 2 * BS:4 * BS], AF.Tanh)
            nc.vector.tensor_mul(hdst[:], so[:], tcb[:])

        assert t_steps % unroll == 0 and unroll % 2 == 0
        with tc.For_i(0, t_steps * BS, unroll * BS,
                      hint_engines=(mybir.EngineType.PE,)) as xoff:
            xt = xpool.tile([7, unroll * BS], BF16, tag="xt")
            nc.sync.dma_start(xt[:], xa_d[:, ds(xoff, unroll * BS)])
            for u in range(unroll):
                if u % 2 == 0:
                    step(xt, u, hA, cA, hB, cB)
                else:
                    step(xt, u, hB, cB, hA, cA)

        # heads: out[b, j] = sum_k h[b, k] * Wcat[j, k] + bias[j]
        ph = psum.tile([BS, 10], F32, tag="ph")
        nc.tensor.matmul(ph[:], hA[:, 0:BS], whd_sb[:, 0:10],
                         start=True, stop=False)
        nc.tensor.matmul(ph[:], hA[:, BS:2 * BS], whd_sb[:, 10:20],
                         start=False, stop=True)
        outsb = work.tile([BS, 10], F32, tag="outsb")
        nc.vector.tensor_add(outsb[:], ph[:], bhd_sb[:])
        nc.sync.dma_start(out_d[:], outsb[:])

    nc.compile()
    return nc


def prep_weights(W_ih, W_hh, b_ih, b_hh, W_gender, b_gender, W_hand, b_hand,
                 W_years, b_years, W_level, b_level):
    W_ih = np.asarray(W_ih, np.float32)
    W_hh = np.asarray(W_hh, np.float32)
    bias = np.asarray(b_ih, np.float32) + np.asarray(b_hh, np.float32)

    Whh_p = W_hh[PERM, :]  # [1024, 256]
    # whh[k, (2c+kc)*128 + m] = Whh_p[128c+m, 128kc+k]
    whh = (Whh_p.reshape(NCH, 128, 2, 128).transpose(3, 0, 2, 1)
           .reshape(128, 16 * 128).astype(NPBF16))
    wxb = np.empty((7, NCH * 128), np.float32)
    wxb[0:6] = W_ih[PERM, :].T
    wxb[6] = bias[PERM]
    wxb = wxb.astype(NPBF16)

    Wcat = np.vstack([np.asarray(W_gender, np.float32),
                      np.asarray(W_hand, np.float32),
                      np.asarray(W_years, np.float32),
                      np.asarray(W_level, np.float32)])  # [10, 256]
    whd = np.empty((128, 20), np.float32)
    whd[:, 0:10] = Wcat[:, 0:128].T
    whd[:, 10:20] = Wcat[:, 128:256].T
    whd = whd.astype(NPBF16)
    bcat = np.concatenate([np.asarray(b_gender, np.float32),
                           np.asarray(b_hand, np.float32),
                           np.asarray(b_years, np.float32),
                           np.asarray(b_level, np.float32)])
    bhd = np.tile(bcat[None, :], (BS, 1)).astype(np.float32)
    return whh, wxb, whd, bhd


def prep_x(x_shard, t_steps):
    """x_shard [BS, t, 6] -> xa [7, t*BS] bf16 with constant-1 row."""
    xs = np.asarray(x_shard, np.float32)
    xa = np.empty((7, t_steps * BS), np.float32)
    xa[0:6] = xs.transpose(2, 1, 0).reshape(6, t_steps * BS)
    xa[6] = 1.0
    return xa.astype(NPBF16)


_CACHE = {}


def _get_nc():
    if "nc" not in _CACHE:
        _CACHE["nc"] = build()
    return _CACHE["nc"]


def kernel(x, W_ih, W_hh, b_ih, b_hh, W_gender, b_gender, W_hand, b_hand,
           W_years, b_years, W_level, b_level, _trace=False):
    nc = _get_nc()
    whh, wxb, whd, bhd = prep_weights(
        W_ih, W_hh, b_ih, b_hh, W_gender, b_gender, W_hand, b_hand,
        W_years, b_years, W_level, b_level)
    x = np.asarray(x, np.float32)
    in_maps = []
    for c in range(NCORES):
        xa = prep_x(x[c * BS:(c + 1) * BS], T)
        in_maps.append({"whh": whh, "wxb": wxb, "xa": xa, "whd": whd,
                        "bhd": bhd})
    res = run_bass_kernel_spmd(nc, in_maps, core_ids=list(range(NCORES)),
                               trace=_trace)
    _CACHE["last_results"] = res
    full = np.concatenate([res.results[i]["out"] for i in range(NCORES)],
                          axis=0).astype(np.float32)
    return (full[:, 0:2], full[:, 2:4], full[:, 4:7], full[:, 7:10])
